# revision 11
# baseline (speedup 1.0000x reference)
"""Trainium2 Bass kernel for nn_BasicNCAModel (neural cellular automaton).

Model (per step, 4 steps):
  y = concat([x, dwconv3x3(x, f1), dwconv3x3(x, f2)])   (reflect pad)
  dx = relu(y @ w1 + b1) @ w2
  x  = x + dx * (stoch > 0.5) * ch_mask

Kernel strategy (fp8 DoubleRow edition):
  - Pure data parallel: batch 16 -> 2 samples on each of 8 NeuronCores.
  - Channel-major layout [C=32, H, W]; the depthwise convs + first dense
    layer fold into a 3x3 conv with effective weights
    W_eff[dy,dx] = diag(f1[dy,dx]) @ w1[32:64] + diag(f2[dy,dx]) @ w1[64:96]
    (+ w1[0:32] folded into the center tap).  x is loaded as 3 vertically
    shifted fp8(e4m3) copies stacked on partitions (K=96); horizontal taps
    are free-dim AP offsets.
  - fp8 MatmulPerfMode.DoubleRow processes 2 K-subtiles (K=192) per pass at
    full column rate.  The 2 subtiles come from the SAME x tile via an
    overlapping-stride AP (subtile dim stride = 1 column), so one DR pass
    covers horizontal taps dx=0 (s0) and dx=1 (s1).
  - A DR matmul must NOT be the last writer of a PSUM accumulation group
    (hardware wedges otherwise), so each group is closed by a REGULAR fp8
    matmul carrying the dx=2 taps (K=96) plus, on partitions 96:128, a
    weight-quantization correction: an "id lane" holding x8hi (col-shifted
    +2) multiplied by Wlo = q8(S_W*Wc - q8(S_W*Wc)), which restores most of
    the precision lost by quantizing the folded center weights to fp8.
  - Per 512-pixel tile: (DR + closer) x 2 output halves + 2 fp16 K=128
    passes for layer 2 = 6 PE passes (vs 8 for pure fp16).
  - b1 bias + 1/S_W descale folded into the relu activation (per-half bias
    APs).  ch_mask folded into w2 (cols 0..2 zeroed, fp16).
  - Fire mask precomputed on host as {0,1} f32, loaded per band with a DMA
    partition-broadcast (no gpsimd broadcast op).
  - Residual path exact fp32: xc loaded windowed-contiguous, xn stored
    windowed; all DVE ops contiguous.
  - Band loads are software-prefetched one band iteration ahead on the
    gpsimd queue; x ping-pongs between two DRAM buffers across steps.
"""

import numpy as np
from contextlib import ExitStack

import concourse.bacc as bacc
import concourse.tile as tile
from concourse import mybir
from concourse.ap import AP
from concourse.bass_utils import run_bass_kernel_spmd

F32 = mybir.dt.float32
F16 = mybir.dt.float16
F8 = mybir.dt.float8e4
AF = mybir.ActivationFunctionType
OP = mybir.AluOpType
PM = mybir.MatmulPerfMode

B, C, H, W = 16, 32, 256, 256
IMG = 3
NCORES = 8
BPC = B // NCORES          # samples per core = 2
BR = 16                    # band rows
NB = H // BR               # bands per sample = 16
RPT = 2                    # rows per matmul tile
TPB = BR // RPT            # tiles per band = 8
NSTEP = 4
WP = W + 2                 # padded row length 258
S_W = 64.0                 # fp8 weight scale


def _seg_rows(r0: int, dy: int):
    """Contiguous (src_row, dst_row, n) segments for one vertical copy,
    with reflect handling at the image top/bottom (reflect: -1->1, 256->254)."""
    rows = [r0 + dy + i for i in range(BR)]
    refl = [(-r if r < 0 else (2 * (H - 1) - r if r > H - 1 else r)) for r in rows]
    segs = []
    i = 0
    while i < BR:
        j = i + 1
        while j < BR and refl[j] == refl[i] + (j - i):
            j += 1
        segs.append((refl[i], i, j - i))
        i = j
    return segs


def _build():
    nc = bacc.Bacc("TRN2", target_bir_lowering=False, debug=False,
                   num_devices=NCORES)
    xin = nc.dram_tensor("xin", [BPC, C, H, WP], F32, kind="ExternalInput").ap()
    maskd = nc.dram_tensor("maskd", [NSTEP, BPC, H, W], F32,
                           kind="ExternalInput").ap()
    wdr1 = nc.dram_tensor("wdr1", [96, 512], F8, kind="ExternalInput").ap()
    wcl = nc.dram_tensor("wcl", [128, 256], F8, kind="ExternalInput").ap()
    w2h = nc.dram_tensor("w2h", [128, 256], F16, kind="ExternalInput").ap()
    bia = nc.dram_tensor("bia", [128, 2], F32, kind="ExternalInput").ap()
    yout = nc.dram_tensor("y", [BPC, C, H, WP], F32, kind="ExternalOutput").ap()

    with tile.TileContext(nc) as tc, ExitStack() as ctx:
        dram = ctx.enter_context(tc.tile_pool(name="dram", bufs=1, space="DRAM"))
        xA = dram.tile([BPC, C, H, WP], F32, name="xA")
        xB = dram.tile([BPC, C, H, WP], F32, name="xB")

        wpool = ctx.enter_context(tc.tile_pool(name="wpool", bufs=1))
        w1t = wpool.tile([96, 512], F8, name="w1t")
        wclt = wpool.tile([128, 256], F8, name="wclt")
        w2t = wpool.tile([128, 256], F16, name="w2t")
        bt = wpool.tile([128, 2], F32, name="bt")
        nc.sync.dma_start(w1t[:], wdr1)
        nc.sync.dma_start(wclt[:], wcl)
        nc.sync.dma_start(w2t[:], w2h)
        nc.sync.dma_start(bt[:], bia)

        xt_pool = ctx.enter_context(tc.tile_pool(name="xt", bufs=3))
        xc_pool = ctx.enter_context(tc.tile_pool(name="xc", bufs=2))
        stb_pool = ctx.enter_context(tc.tile_pool(name="stb", bufs=2))
        xn_pool = ctx.enter_context(tc.tile_pool(name="xn", bufs=2))
        hs_pool = ctx.enter_context(tc.tile_pool(name="hs", bufs=3))
        dxm_pool = ctx.enter_context(tc.tile_pool(name="dxm", bufs=3))
        hp_pool = ctx.enter_context(tc.tile_pool(name="hp", bufs=3, space="PSUM"))
        dxp_pool = ctx.enter_context(tc.tile_pool(name="dxp", bufs=2, space="PSUM"))

        pp_bufs = [xA[:], xB[:]]
        srcs = [xin if i == 0 else pp_bufs[(i - 1) % 2] for i in range(NSTEP)]
        dsts = [yout if i == NSTEP - 1 else pp_bufs[i % 2] for i in range(NSTEP)]

        items = [(st, s, b) for st in range(NSTEP) for s in range(BPC)
                 for b in range(NB)]

        def emit_loads(it):
            step, s, b = it
            src = srcs[step]
            r0 = b * BR
            # --- conv region: 3 vertically shifted fp8 copies on partitions
            # [dy=-1 (0:32), dy=0 (32:64), dy=+1 (64:96)]; id lane 96:128 ---
            xt = xt_pool.tile([128, BR * WP], F8)
            pp = xt[:].ap[0][0]
            xtr = AP(tensor=xt[:].tensor, offset=xt[:].offset,
                     ap=[[pp, 128], [WP, BR], [1, WP]])
            if 0 < b < NB - 1:
                # single merged trigger: src rows r0-1 .. r0+BR contiguous,
                # dy stride WP, channel stride H*WP
                src_ap = src[s]  # [C, H, WP]
                sap = AP(tensor=src_ap.tensor,
                         offset=src_ap.offset + (r0 - 1) * WP,
                         ap=[[WP, 3], [H * WP, 32], [1, BR * WP]])
                dst = AP(tensor=xt[:].tensor, offset=xt[:].offset,
                         ap=[[pp, 96], [1, BR * WP]])
                nc.gpsimd.dma_start(dst, sap)
            else:
                for gi, dy in enumerate((-1, 0, 1)):
                    p0 = gi * 32
                    for (sr, dr, n) in _seg_rows(r0, dy):
                        nc.gpsimd.dma_start(
                            xtr[p0:p0 + 32, dr:dr + n, :],
                            src[s, :, sr:sr + n, :])
            # --- id lane: x8hi (dy=0 interior) at column shift +2, so the
            # closer pass (base+2) reads x8hi[row, col c] for out col c ---
            nc.gpsimd.dma_start(xtr[96:128, :, 2:W + 2],
                                xtr[32:64, :, 1:W + 1])

            # --- exact fp32 interior window for the residual add ---
            xc = xc_pool.tile([32, BR * W], F32)
            nc.gpsimd.dma_start(
                xc[:].rearrange("p (r c) -> p r c", c=W),
                src[s, :, r0:r0 + BR, 1:W + 1])

            # --- fire mask, partition-broadcast from DRAM ---
            stb = stb_pool.tile([32, BR * W], F32)
            nc.sync.dma_start(
                stb[:],
                maskd[step, s, r0:r0 + BR, :].flatten().unsqueeze(0)
                .partition_broadcast(32))
            return (xt, xc, stb)

        def emit_compute(it, tiles):
            step, s, b = it
            dst = dsts[step]
            r0 = b * BR
            xt, xc, stb = tiles
            pp = xt[:].ap[0][0]
            xtr = AP(tensor=xt[:].tensor, offset=xt[:].offset,
                     ap=[[pp, 128], [WP, BR], [1, WP]])
            # reflect column pads (conv partitions only):
            # col0 <- col2, col257 <- col255
            nc.vector.tensor_copy(xtr[0:96, :, 0:1], xtr[0:96, :, 2:3])
            nc.vector.tensor_copy(xtr[0:96, :, WP - 1:WP],
                                  xtr[0:96, :, WP - 3:WP - 2])

            xcr = xc[:].rearrange("p (r c) -> p r c", c=W)
            stbr = stb[:].rearrange("p (r c) -> p r c", c=W)
            xn = xn_pool.tile([32, BR * W], F32)
            xnr = xn[:].rearrange("p (r c) -> p r c", c=W)

            w1v = w1t[:].rearrange("k (h s m) -> k h s m", h=2, s=2)
            wcv = wclt[:].rearrange("k (h m) -> k h m", h=2)

            pend = None  # (hs, rt) awaiting layer 2
            for t in range(TPB + 1):
                if t < TPB:
                    rt = t * RPT
                    base = xt[:].offset + rt * WP
                    rhs1 = AP(tensor=xt[:].tensor, offset=base,
                              ap=[[pp, 96], [1, 2], [WP, RPT], [1, W]])
                    rhscl = AP(tensor=xt[:].tensor, offset=base + 2,
                               ap=[[pp, 128], [WP, RPT], [1, W]])
                    hps = [hp_pool.tile([128, 512], F32, name=f"hp{h}")
                           for h in range(2)]
                    for h in range(2):
                        out = hps[h][:]
                        nc.tensor.matmul(out, w1v[:, h], rhs1,
                                         start=True, stop=False,
                                         perf_mode=PM.DoubleRow)
                        nc.tensor.matmul(out, wcv[:, h], rhscl,
                                         start=False, stop=True)
                    hs = hs_pool.tile([128, 1024], F16)
                    for h in range(2):
                        nc.scalar.activation(hs[:, h * 512:(h + 1) * 512],
                                             hps[h][:],
                                             AF.Relu, bias=bt[:, h:h + 1],
                                             scale=1.0 / S_W)
                    pend, prev = (hs, rt), pend
                else:
                    prev, pend = pend, None
                if prev is None:
                    continue
                hs_p, rp = prev
                # ---- layer 2: dx = h @ w2 (fp16, K=256 split in two).
                # M padded to 128 with zero cols: col-32 PE tiling mixed with
                # DoubleRow matmuls + concurrent SWDGE loads wedges the HW ----
                dxp = dxp_pool.tile([128, 512], F32)
                nc.tensor.matmul(dxp[:], w2t[:, 0:128], hs_p[:, 0:512],
                                 start=True, stop=False)
                nc.tensor.matmul(dxp[:], w2t[:, 128:256], hs_p[:, 512:1024],
                                 start=False, stop=True)
                # ---- masked residual: xn = mask*dx + x (all contiguous) ----
                dxm = dxm_pool.tile([32, RPT * W], F32)
                nc.vector.tensor_tensor(
                    dxm[:], dxp[0:32, :],
                    stbr[:, rp:rp + RPT, :].rearrange("p r c -> p (r c)"),
                    op=OP.mult)
                nc.vector.tensor_tensor(
                    xnr[:, rp:rp + RPT, :], dxm[:]
                    .rearrange("p (r c) -> p r c", c=W),
                    xcr[:, rp:rp + RPT, :], op=OP.add)

            # ---- store band interior (pads untouched) ----
            nc.sync.dma_start(
                dst[s, :, r0:r0 + BR, 1:W + 1],
                xnr)

        pending = None
        for idx in range(len(items) + 1):
            loaded = emit_loads(items[idx]) if idx < len(items) else None
            if pending is not None:
                emit_compute(items[idx - 1], pending)
            pending = loaded
    nc.compile()
    return nc


_NC_CACHE = None


def _get_nc():
    global _NC_CACHE
    if _NC_CACHE is None:
        _NC_CACHE = _build()
    return _NC_CACHE


def _q8(a):
    import ml_dtypes
    return np.asarray(a, np.float32).astype(ml_dtypes.float8_e4m3fn)


def _make_in_maps(x, f1, f2, w1, b1, w2, stoch):
    f1 = np.asarray(f1, np.float64)[:, :, 0, :]   # [3,3,32]
    f2 = np.asarray(f2, np.float64)[:, :, 0, :]
    w1 = np.asarray(w1, np.float64)               # [96,256]
    b1 = np.asarray(b1, np.float32)               # [256]
    w2 = np.asarray(w2, np.float64).copy()        # [256,32]
    w2[:, :IMG] = 0.0                             # ch_mask folded into w2

    # conv-only effective weights; identity folded into the center tap
    weff = (f1[:, :, :, None] * w1[None, None, 32:64, :]
            + f2[:, :, :, None] * w1[None, None, 64:96, :])   # [3,3,32,256]
    Wc = weff[1, 1] + w1[0:32]                                # folded center

    def tapcol(dy, dx):  # [32,256] f64, partition row order dy=-1,0,+1
        if dy == 1 and dx == 1:
            return Wc
        return weff[dy, dx]

    def stack_dy(dx):    # [96, 256]
        return np.concatenate([tapcol(0, dx), tapcol(1, dx), tapcol(2, dx)],
                              axis=0)

    s0 = _q8(S_W * stack_dy(0)).astype(np.float32)    # DR1 s0: dx=0 taps
    s1 = _q8(S_W * stack_dy(1)).astype(np.float32)    # DR1 s1: dx=1 (+id fold)
    # wdr1: [96, h, s, 128] -> [96, 512] fp8
    wdr1 = np.zeros((96, 2, 2, 128), np.float32)
    for h in range(2):
        wdr1[:, h, 0, :] = s0[:, h * 128:(h + 1) * 128]
        wdr1[:, h, 1, :] = s1[:, h * 128:(h + 1) * 128]
    wdr1 = _q8(wdr1.reshape(96, 512))

    # closer: rows 0:96 = dx=2 taps; rows 96:128 = Wlo (center-weight
    # quantization correction, multiplied by the x8hi id lane)
    c2 = _q8(S_W * stack_dy(2)).astype(np.float32)    # [96,256]
    Wc8 = _q8(S_W * Wc).astype(np.float32)
    Wlo = _q8(S_W * np.asarray(Wc, np.float64) - Wc8).astype(np.float32)
    wcl = np.zeros((128, 2, 128), np.float32)
    for h in range(2):
        wcl[0:96, h, :] = c2[:, h * 128:(h + 1) * 128]
        wcl[96:128, h, :] = Wlo[:, h * 128:(h + 1) * 128]
    wcl = _q8(wcl.reshape(128, 256))

    w2h = np.zeros((128, 2, 128), np.float64)
    w2h[:, 0, 0:32] = w2[0:128, :]
    w2h[:, 1, 0:32] = w2[128:256, :]
    w2h = w2h.reshape(128, 256).astype(np.float16)
    bia = np.stack([b1[0:128], b1[128:256]], axis=1).astype(np.float32)  # [128,2]

    x = np.asarray(x, np.float32)
    maskf = (np.asarray(stoch, np.float32)[:, :, :, :, 0] > 0.5).astype(np.float32)
    in_maps = []
    for i in range(NCORES):
        xi = np.transpose(x[i * BPC:(i + 1) * BPC], (0, 3, 1, 2))  # [2,32,H,W]
        xpad = np.zeros((BPC, C, H, WP), np.float32)
        xpad[:, :, :, 1:W + 1] = xi
        mi = np.ascontiguousarray(maskf[:, i * BPC:(i + 1) * BPC])
        in_maps.append({"xin": xpad, "maskd": mi,
                        "wdr1": wdr1.view(np.uint8),
                        "wcl": wcl.view(np.uint8),
                        "w2h": w2h, "bia": bia})
    return in_maps


def kernel(x, f1, f2, w1, b1, w2, stoch, steps):
    assert int(steps) == NSTEP, f"kernel compiled for {NSTEP} steps, got {steps}"
    nc = _get_nc()
    in_maps = _make_in_maps(x, f1, f2, w1, b1, w2, stoch)
    res = run_bass_kernel_spmd(nc, in_maps, core_ids=list(range(NCORES)))
    outs = []
    for i in range(NCORES):
        yi = res.results[i]["y"][:, :, :, 1:W + 1]     # strip col pads
        outs.append(np.transpose(yi, (0, 2, 3, 1)))    # -> [2,256,256,32]
    return np.ascontiguousarray(np.concatenate(outs, axis=0)).astype(np.float32)
